# revision 1
# baseline (speedup 1.0000x reference)
"""Single-head attention (B=4, L=4096, EMB=312, HID=256) on 8 NeuronCores.

Sharding: data-parallel over batch (4) x key-parallel (2) = 8 cores. Each
core handles ALL 4096 queries against its half of the keys and returns the
UNNORMALIZED partial [sum_k p*v | sum_k p] rows; the host combines the two
halves as (o1+o2)/(s1+s2). Key-sharding (vs query-sharding) halves the
duplicated K/V projection work; only the Q projection is duplicated.

Per-core device algorithm:
  - Host sends transposed, padded inputs split into bf16 (hi, lo) pairs; a
    matmul A@B is computed as A_hi@B_hi + A_lo@B_hi + A_hi@B_lo (the dropped
    lo@lo term is ~2^-18 relative), giving ~fp32-quality products at the
    bf16 PE rate (1 cycle/row).
  - embT carries a ones-row at index EMB and W* carry the bias in that row,
    so projections fold the bias in. Wv has 2 extra columns: ones (gives the
    softmax row-sum through the P@V matmul) and zero padding (even N).
  - Scores are computed transposed: sT[kl, ql] = kT-chunk^T @ qT, so the
    exp() output is directly the stationary operand for the P@V matmul —
    no on-device transposes anywhere.
  - Mask is host-side transposed, scaled by -1e5, cast to bf16; applied
    additively to the score PSUM by the vector engine. exp() on the scalar
    engine. The raw partials (P@V columns + row-sum column) go back to the
    host, which normalizes after combining the key-halves.

Precision ladder (BASS_KERNEL_PRECISION); projections are always bf16x2.
Measured absmax error relative to max|output| and TimelineSim time/core:
  - "fast":     fp32r single-pass QK and PV          ~9.4e-4 of scale
  - "fp16qk":   fp16 single-pass QK, fp32r PV        ~1.6e-3 of scale
  - "balanced": bf16x2 3-term QK, fp32r PV           ~1.6e-4, ~312 us  (default)
  - "exact":    also bf16x2 p/v in PV                ~3.8e-5, ~440 us
"""
import os

import numpy as np
import ml_dtypes

import concourse.bacc as bacc
import concourse.tile as tile
from concourse import mybir, bass2jax
from concourse.bass_utils import run_bass_kernel_spmd

# Debug aid (opt-in): surface real compile errors from the PJRT compile
# hook, which the C++ bridge otherwise swallows.
if os.environ.get("BASS_KERNEL_DEBUG"):
    import functools as _ft
    import traceback as _tb
    _orig_hook = bass2jax.neuronx_cc_hook
    @_ft.wraps(_orig_hook)
    def _dbg_hook(*args, **kwargs):
        try:
            return _orig_hook(*args, **kwargs)
        except BaseException:
            _tb.print_exc()
            raise
    bass2jax.neuronx_cc_hook = _dbg_hook

EMB, HID, B, L = 312, 256, 4, 4096
NCORES = 8
P = 128
KL = L // 2            # key rows per core (key-parallel halves)
EPAD = 384             # emb dim padded to 3 partition chunks; row EMB is the ones-row
HV = HID + 2           # v columns: HID values | ones | zero pad (even N for matmul)
QT = 512               # ql tile width (PSUM bank = 512 fp32)
NKC = KL // P          # 16 kl chunks per core
NQTT = L // QT         # 8 ql tiles per core (all queries)
NKT = KL // QT         # 4 l tiles for the k projection
MASK_SCALE = np.float32(-100000.0)

F32 = mybir.dt.float32
F16 = mybir.dt.float16
F32R = mybir.dt.float32r
BF16 = mybir.dt.bfloat16
BF = ml_dtypes.bfloat16

_CACHE = {}

# (lhs_piece, rhs_piece) index pairs for the 3-term bf16x2 product.
SPLIT3 = ((0, 0), (1, 0), (0, 1))


def _build(precision):
    qk_exact = precision in ("balanced", "exact")
    qk_fp16 = precision == "fp16qk"
    pv_exact = precision == "exact"

    nc = bacc.Bacc(None)

    def dram_pair(name, shape):
        return tuple(
            nc.dram_tensor(f"{name}{s}", shape, BF16, kind="ExternalInput")
            for s in ("_hi", "_lo")
        )

    embT = dram_pair("embT", [EPAD, L])
    embTk = dram_pair("embTk", [EPAD, KL])
    wq = dram_pair("wq", [EPAD, HID])
    wk = dram_pair("wk", [EPAD, HID])
    wv = dram_pair("wv", [EPAD, HV])
    maskT = nc.dram_tensor("maskT", [KL, L], BF16, kind="ExternalInput")
    out = nc.dram_tensor("out", [L, HID + 1], F32, kind="ExternalOutput")

    with tile.TileContext(nc) as tc:
        with (
            tc.tile_pool(name="big", bufs=1) as big,
            tc.tile_pool(name="wp", bufs=1) as wp,
            tc.tile_pool(name="mt", bufs=10) as mtp,
            tc.tile_pool(name="pt", bufs=4) as ptp,
            tc.tile_pool(name="fin", bufs=4) as fin,
            tc.tile_pool(name="ps_st", bufs=4, space="PSUM") as ps_st,
            tc.tile_pool(name="ps_pv", bufs=1, space="PSUM") as ps_pv,
        ):
            # ---- load inputs (as [P, chunk, free] with the chunk index in
            # the free dim; partition line p reads rows {p, 128+p, 256+p}).
            # Large tensors are loaded in column blocks, lowest columns first
            # across all chunks, so the first projection matmuls can start
            # ~2us in instead of waiting for the whole 6 MB transfer.
            def load_pair(pool, name, dram, ncol, blk=None):
                ts = [
                    pool.tile([P, 3, ncol], BF16, name=f"{name}_{s}", tag=f"{name}_{s}")
                    for s in ("hi", "lo")
                ]
                if blk is None:
                    # Small (weight) loads ride the second HWDGE ring (ACT)
                    # so they don't serialize ahead of the first embTk
                    # blocks on the SP ring at startup.
                    for t, d in zip(ts, dram):
                        nc.scalar.dma_start(out=t, in_=d[:, :].rearrange("(c p) n -> p c n", p=P))
                else:
                    # hi and lo interleaved per column block: the 3-term
                    # projection of block b needs both pieces of block b.
                    for b0 in range(0, ncol, blk):
                        for c in range(3):
                            for t, d in zip(ts, dram):
                                nc.sync.dma_start(
                                    out=t[:, c, b0:b0 + blk],
                                    in_=d[c * P:(c + 1) * P, b0:b0 + blk],
                                )
                return tuple(ts)

            wq_t = load_pair(wp, "wq", wq, HID)
            wk_t = load_pair(wp, "wk", wk, HID)
            wv_t = load_pair(wp, "wv", wv, HV)
            # embTk first: the projection phase starts with k/v tiles,
            # which consume the key-half slice.
            embTk_t = load_pair(big, "embTk", embTk, KL, blk=QT)
            embT_t = load_pair(big, "embT", embT, L, blk=QT)

            def mm3(ps, lhs_pair, rhs_pair, lslice, rslice):
                """ps = sum over 3 e-chunks of (lhs @ rhs) in bf16x2 3-term form."""
                n = len(SPLIT3) * 3
                i = 0
                for a, b in SPLIT3:
                    for e in range(3):
                        nc.tensor.matmul(
                            ps,
                            lhsT=lhs_pair[a][(slice(None), e) + lslice],
                            rhs=rhs_pair[b][(slice(None), e) + rslice],
                            start=(i == 0), stop=(i == n - 1),
                        )
                        i += 1

            # ---- projections
            # q/k in [h(part), hc, l(free)] layout; v in [kl(part), klc, h] layout.
            if qk_exact:
                kT_h = big.tile([P, 2, KL], BF16, name="kT_h")
                kT_l = big.tile([P, 2, KL], BF16, name="kT_l")
                qT_h = big.tile([P, 2, L], BF16, name="qT_h")
                qT_l = big.tile([P, 2, L], BF16, name="qT_l")
            elif qk_fp16:
                kT_r = big.tile([P, 2, KL], F16, name="kT_r")
                qT_r = big.tile([P, 2, L], F16, name="qT_r")
            else:
                kT_r = big.tile([P, 2, KL], F32R, name="kT_r")
                qT_r = big.tile([P, 2, L], F32R, name="qT_r")
            if pv_exact:
                v_h = big.tile([P, NKC, HV], BF16, name="v_h")
                v_l = big.tile([P, NKC, HV], BF16, name="v_l")
            else:
                v_r = big.tile([P, NKC, HV], F32R, name="v_r")

            def split_store(ps, hi_ap, lo_ap):
                nc.scalar.copy(out=hi_ap, in_=ps)
                nc.vector.tensor_sub(lo_ap, ps, hi_ap)

            def emit_kq(hc, lt, which):
                ps = ps_st.tile([P, QT], F32, name="st", tag="st")
                w, e, dsts = (
                    (wk_t, embTk_t, (kT_h, kT_l) if qk_exact else (kT_r,))
                    if which == "k"
                    else (wq_t, embT_t, (qT_h, qT_l) if qk_exact else (qT_r,))
                )
                mm3(ps, w, e, (slice(hc * P, (hc + 1) * P),),
                    (slice(lt * QT, (lt + 1) * QT),))
                dst = (slice(None), hc, slice(lt * QT, (lt + 1) * QT))
                if qk_exact:
                    split_store(ps, dsts[0][dst], dsts[1][dst])
                else:
                    nc.scalar.copy(out=dsts[0][dst], in_=ps)

            def emit_v(kc):
                ps = ps_st.tile([P, QT], F32, name="st", tag="st")
                mm3(ps[:, :HV], embTk_t, wv_t, (slice(kc * P, (kc + 1) * P),),
                    (slice(None),))
                dst = (slice(None), kc, slice(None))
                if pv_exact:
                    split_store(ps[:, :HV], v_h[dst], v_l[dst])
                else:
                    nc.scalar.copy(out=v_r[dst], in_=ps[:, :HV])

            # Interleave the k/q tiles (PSUM->SBUF copy has slack) with the
            # v tiles (copy-bound) so the scalar/vector copies never gate PE.
            kq_tiles = [("k", hc, lt) for hc in range(2) for lt in range(NKT)]
            kq_tiles += [("q", hc, lt) for hc in range(2) for lt in range(NQTT)]
            vi = 0
            for i, (which, hc, lt) in enumerate(kq_tiles):
                emit_kq(hc, lt, which)
                want_v = ((i + 1) * NKC) // len(kq_tiles)
                while vi < want_v:
                    emit_v(vi)
                    vi += 1
            while vi < NKC:
                emit_v(vi)
                vi += 1

            # ---- attention
            # Software-pipelined emission: chunk kc's P@V matmuls are emitted
            # AFTER chunk kc+1's QK matmuls, so the PE always has independent
            # work in program order while the DVE mask-add + ACT exp of the
            # current chunk are still in flight.
            for qt in range(NQTT):
                pvs = [
                    ps_pv.tile([P, HV], F32, name=f"pv{j}", tag=f"pv{j}")
                    for j in range(4)
                ]
                qsl = slice(qt * QT, (qt + 1) * QT)
                pending_pv = None  # (kc, p-tiles) awaiting PV emission

                def emit_pv(kc, ptile):
                    for j in range(4):
                        jsl = slice(j * P, (j + 1) * P)
                        if pv_exact:
                            for t, (a, b) in enumerate(SPLIT3):
                                nc.tensor.matmul(
                                    pvs[j],
                                    lhsT=ptile[a][:, jsl],
                                    rhs=(v_h, v_l)[b][:, kc, :],
                                    start=(kc == 0 and t == 0),
                                    stop=(kc == NKC - 1 and t == 2),
                                )
                        else:
                            nc.tensor.matmul(
                                pvs[j],
                                lhsT=ptile[:, jsl],
                                rhs=v_r[:, kc, :],
                                start=(kc == 0), stop=(kc == NKC - 1),
                            )

                for kc in range(NKC):
                    ksl = slice(kc * P, (kc + 1) * P)
                    st = ps_st.tile([P, QT], F32, name="st", tag="st")
                    if qk_exact:
                        kp, qp = (kT_h, kT_l), (qT_h, qT_l)
                        n = 2 * len(SPLIT3)
                        i = 0
                        for a, b in SPLIT3:
                            for hc in range(2):
                                nc.tensor.matmul(
                                    st,
                                    lhsT=kp[a][:, hc, ksl],
                                    rhs=qp[b][:, hc, qsl],
                                    start=(i == 0), stop=(i == n - 1),
                                )
                                i += 1
                    else:
                        for hc in range(2):
                            nc.tensor.matmul(
                                st,
                                lhsT=kT_r[:, hc, ksl],
                                rhs=qT_r[:, hc, qsl],
                                start=(hc == 0), stop=(hc == 1),
                            )
                    if pending_pv is not None:
                        emit_pv(*pending_pv)
                    mt = mtp.tile([P, QT], BF16, name="mt", tag="mt")
                    nc.sync.dma_start(out=mt, in_=maskT[ksl, qsl])
                    nc.vector.tensor_tensor(out=st, in0=st, in1=mt, op=mybir.AluOpType.add)
                    if pv_exact:
                        pe = ptp.tile([P, QT], F32, name="pe", tag="pe")
                        nc.scalar.activation(out=pe, in_=st, func=mybir.ActivationFunctionType.Exp)
                        p_h = ptp.tile([P, QT], BF16, name="p_h", tag="p_h")
                        p_l = ptp.tile([P, QT], BF16, name="p_l", tag="p_l")
                        nc.vector.tensor_copy(p_h, pe)
                        nc.gpsimd.tensor_sub(p_l, pe, p_h)
                        pending_pv = (kc, (p_h, p_l))
                    else:
                        pt = ptp.tile([P, QT], F32R, name="pt", tag="pt")
                        nc.scalar.activation(out=pt, in_=st, func=mybir.ActivationFunctionType.Exp)
                        pending_pv = (kc, pt)
                emit_pv(*pending_pv)
                for j in range(4):
                    # Ship the unnormalized partial [sum p*v | sum p]; the
                    # host divides after combining the two key-halves.
                    ot = fin.tile([P, HID + 1], F32, name="ot", tag="ot")
                    nc.vector.tensor_copy(ot, pvs[j][:, :HID + 1])
                    row0 = (qt * 4 + j) * P
                    nc.sync.dma_start(out=out[row0:row0 + P, :], in_=ot)
    nc.finalize()
    return nc


def _get_nc():
    precision = os.environ.get("BASS_KERNEL_PRECISION", "balanced")
    key = f"nc_{precision}"
    if key not in _CACHE:
        _CACHE[key] = _build(precision)
    return _CACHE[key]


def _split_pair(x):
    hi = x.astype(BF)
    lo = (x - hi.astype(np.float32)).astype(BF)
    return hi, lo


def kernel(embedding, mask, Wq, bq, Wk, bk, Wv, bv):
    embedding = np.asarray(embedding, dtype=np.float32)
    mask = np.asarray(mask, dtype=np.float32)
    Wq = np.asarray(Wq, dtype=np.float32)
    Wk = np.asarray(Wk, dtype=np.float32)
    Wv = np.asarray(Wv, dtype=np.float32)
    bq = np.asarray(bq, dtype=np.float32)
    bk = np.asarray(bk, dtype=np.float32)
    bv = np.asarray(bv, dtype=np.float32)

    def pad_w(w, b, extra_one=False):
        wp = np.zeros((EPAD, HV if extra_one else HID), dtype=np.float32)
        wp[:EMB, :HID] = w
        wp[EMB, :HID] = b
        if extra_one:
            wp[EMB, HID] = 1.0
        return wp

    wq_h, wq_l = _split_pair(pad_w(Wq, bq))
    wk_h, wk_l = _split_pair(pad_w(Wk, bk))
    wv_h, wv_l = _split_pair(pad_w(Wv, bv, extra_one=True))

    in_maps = []
    for c in range(NCORES):
        b, half = divmod(c, 2)
        embT = np.zeros((EPAD, L), dtype=np.float32)
        embT[:EMB] = embedding[b].T
        embT[EMB] = 1.0
        e_h, e_l = _split_pair(embT)
        ksl = slice(half * KL, (half + 1) * KL)
        ek_h = np.ascontiguousarray(e_h[:, ksl])
        ek_l = np.ascontiguousarray(e_l[:, ksl])
        mT = np.ascontiguousarray(mask[b].T[ksl, :])
        mT = (mT * MASK_SCALE).astype(BF)
        in_maps.append({
            "embT_hi": e_h, "embT_lo": e_l,
            "embTk_hi": ek_h, "embTk_lo": ek_l,
            "wq_hi": wq_h, "wq_lo": wq_l,
            "wk_hi": wk_h, "wk_lo": wk_l,
            "wv_hi": wv_h, "wv_lo": wv_l,
            "maskT": mT,
        })

    nc = _get_nc()
    trace = bool(int(os.environ.get("BASS_KERNEL_TRACE", "0")))
    res = run_bass_kernel_spmd(nc, in_maps, core_ids=list(range(NCORES)), trace=trace)
    _CACHE["last_results"] = res

    full = np.empty((B, L, HID), dtype=np.float32)
    for b in range(B):
        r0 = res.results[2 * b]["out"].astype(np.float64)
        r1 = res.results[2 * b + 1]["out"].astype(np.float64)
        num = r0[:, :HID] + r1[:, :HID]
        den = r0[:, HID:] + r1[:, HID:]
        full[b] = (num / den).astype(np.float32)
    return full



# revision 2
# speedup vs baseline: 1.2651x; 1.2651x over previous
"""Single-head attention (B=4, L=4096, EMB=312, HID=256) on 8 NeuronCores.

Sharding: data-parallel over batch (4) x key-parallel (2) = 8 cores. Each
core handles ALL 4096 queries against its half of the keys and returns the
UNNORMALIZED partial [sum_k p*v | sum_k p] rows; the host combines the two
halves as (o1+o2)/(s1+s2). Key-sharding (vs query-sharding) halves the
duplicated K/V projection work; only the Q projection is duplicated.

Device algorithm (per core):
  - Inputs arrive pre-transposed/padded from the host. emb and W* are fp16
    (values are small-range, so fp16's 11-bit mantissa beats bf16 and loads
    half the bytes of fp32); projections are single-pass fp16 matmuls
    accumulated in fp32 PSUM (1 cycle/row on the PE, same rate as bf16).
  - embT carries a ones-row at index EMB and W* carry the bias in that row,
    so projections fold the bias in. Wv has 2 extra columns: ones (gives the
    softmax row-sum through the P@V matmul) and zero padding (even N).
  - q/k/v are stored as fp32r; QK and PV run single-pass fp32r matmuls
    (1 cycle/row at these tile widths, ~tf32 operand precision).
  - Scores are computed transposed: sT[kl, ql] = kT-chunk^T @ qT, so the
    exp() output is directly the stationary operand for the P@V matmul —
    no on-device transposes anywhere.
  - Mask is host-side transposed and encoded as fp8e4m3 {0, -240}: adding
    -240 to a score makes exp() underflow to exactly 0.0 in fp32, which is
    indistinguishable from the reference's -1e5 (no row is fully masked).
    The DVE applies it additively to the score PSUM; exp() on ACT.
  - DMA discipline: the TimelineSim charges ~565-667ns of sequencer time
    per dma_start on the SP/ACT/DVE rings plus a shared-HWDGE hold, so
    transfers are batched: 12 emb block loads + 3 weight loads + 8 mask
    loads (one per query tile, on the gpsimd/SWDGE ring which bypasses
    HWDGE) + 8 output stores. ~31 DMAs total vs 232 in the bf16x2 version.
  - The raw partials (P@V columns + row-sum column) go back to the host,
    which normalizes after combining the key-halves.

Env overrides (debug): BASS_KERNEL_MASK_RING=gpsimd|scalar,
BASS_KERNEL_MASK_DT=f8|bf16.
"""
import os

import numpy as np
import ml_dtypes

import concourse.bacc as bacc
import concourse.tile as tile
from concourse import mybir, bass2jax
from concourse.bass_utils import run_bass_kernel_spmd

# Debug aid (opt-in): surface real compile errors from the PJRT compile
# hook, which the C++ bridge otherwise swallows.
if os.environ.get("BASS_KERNEL_DEBUG"):
    import functools as _ft
    import traceback as _tb
    _orig_hook = bass2jax.neuronx_cc_hook
    @_ft.wraps(_orig_hook)
    def _dbg_hook(*args, **kwargs):
        try:
            return _orig_hook(*args, **kwargs)
        except BaseException:
            _tb.print_exc()
            raise
    bass2jax.neuronx_cc_hook = _dbg_hook

EMB, HID, B, L = 312, 256, 4, 4096
NCORES = 8
P = 128
KL = L // 2            # key rows per core (key-parallel halves)
EPAD = 384             # emb dim padded to 3 partition chunks; row EMB is the ones-row
HV = HID + 2           # v columns: HID values | ones | zero pad (even N)
QT = 512               # ql tile width (PSUM bank = 512 fp32)
NKC = KL // P          # 16 kl chunks per core
NQT = L // QT          # 8 ql tiles per core (all queries)
NKT = KL // QT         # 4 l tiles for the k projection
MASK_VAL = np.float32(-240.0)   # exactly representable in fp8e4m3

F32 = mybir.dt.float32
F16 = mybir.dt.float16
F32R = mybir.dt.float32r
F8 = mybir.dt.float8e4
BF16 = mybir.dt.bfloat16
F16NP = np.float16
F8NP = ml_dtypes.float8_e4m3

_CACHE = {}


def _mask_cfg():
    ring = os.environ.get("BASS_KERNEL_MASK_RING", "gpsimd")
    dt = os.environ.get("BASS_KERNEL_MASK_DT", "f8")
    return ring, dt


def _build():
    mask_ring, mask_dt = _mask_cfg()
    MDT = F8 if mask_dt == "f8" else BF16

    nc = bacc.Bacc(None)

    embT = nc.dram_tensor("embT", [EPAD, L], F16, kind="ExternalInput")
    embTk = nc.dram_tensor("embTk", [EPAD, KL], F16, kind="ExternalInput")
    wq = nc.dram_tensor("wq", [EPAD, HID], F16, kind="ExternalInput")
    wk = nc.dram_tensor("wk", [EPAD, HID], F16, kind="ExternalInput")
    wv = nc.dram_tensor("wv", [EPAD, HV], F16, kind="ExternalInput")
    maskT = nc.dram_tensor("maskT", [KL, L], MDT, kind="ExternalInput")
    out = nc.dram_tensor("out", [L, HID + 1], F32, kind="ExternalOutput")

    with tile.TileContext(nc) as tc:
        with (
            tc.tile_pool(name="big", bufs=1) as big,
            tc.tile_pool(name="wp", bufs=1) as wp,
            tc.tile_pool(name="mt", bufs=3) as mtp,
            tc.tile_pool(name="pt", bufs=4) as ptp,
            tc.tile_pool(name="fin", bufs=2) as fin,
            tc.tile_pool(name="ps_st", bufs=4, space="PSUM") as ps_st,
            tc.tile_pool(name="ps_pv", bufs=1, space="PSUM") as ps_pv,
        ):
            # ---- input loads. Weight tensors ride the ACT ring; emb blocks
            # ride the SP ring, lowest columns first so the first projection
            # matmuls start a couple of us in. Each DMA covers all 3
            # emb-chunks of its column block (partition p reads rows
            # {p, 128+p, 256+p}).
            wk_t = wp.tile([P, 3, HID], F16, name="wk_t")
            wv_t = wp.tile([P, 3, HV], F16, name="wv_t")
            wq_t = wp.tile([P, 3, HID], F16, name="wq_t")
            for t, d in ((wk_t, wk), (wv_t, wv), (wq_t, wq)):
                nc.scalar.dma_start(
                    out=t, in_=d[:, :].rearrange("(c p) n -> p c n", p=P))

            embTk_t = big.tile([P, 3, KL], F16, name="embTk_t")
            embT_t = big.tile([P, 3, L], F16, name="embT_t")
            for t, d, n in ((embTk_t, embTk, KL), (embT_t, embT, L)):
                for b0 in range(0, n, QT):
                    nc.sync.dma_start(
                        out=t[:, :, b0:b0 + QT],
                        in_=d[:, b0:b0 + QT].rearrange("(c p) n -> p c n", p=P),
                    )

            kT_r = big.tile([P, 2, KL], F32R, name="kT_r")
            qT_r = big.tile([P, 2, L], F32R, name="qT_r")
            v_r = big.tile([P, NKC, HV], F32R, name="v_r")

            # ---- projections (single-pass fp16, fp32 PSUM accumulate).
            # q/k in [h(part), hc, l(free)] layout; v in [kl(part), klc, h].
            # k/q PSUM->SBUF copies go to the DVE and v copies to ACT so the
            # copy work never gates the PE during the projection phase.
            def emit_kq(hc, lt, which):
                ps = ps_st.tile([P, QT], F32, name="st", tag="st")
                w, e, dst = (
                    (wk_t, embTk_t, kT_r) if which == "k"
                    else (wq_t, embT_t, qT_r)
                )
                for ei in range(3):
                    nc.tensor.matmul(
                        ps,
                        lhsT=w[:, ei, hc * P:(hc + 1) * P],
                        rhs=e[:, ei, lt * QT:(lt + 1) * QT],
                        start=(ei == 0), stop=(ei == 2),
                    )
                nc.vector.tensor_copy(dst[:, hc, lt * QT:(lt + 1) * QT], ps)

            def emit_v(kc):
                ps = ps_st.tile([P, QT], F32, name="st", tag="st")
                for ei in range(3):
                    nc.tensor.matmul(
                        ps[:, :HV],
                        lhsT=embTk_t[:, ei, kc * P:(kc + 1) * P],
                        rhs=wv_t[:, ei, :],
                        start=(ei == 0), stop=(ei == 2),
                    )
                nc.scalar.copy(out=v_r[:, kc, :], in_=ps[:, :HV])

            kq_tiles = [("k", hc, lt) for lt in range(NKT) for hc in range(2)]
            kq_tiles += [("q", hc, lt) for lt in range(NQT) for hc in range(2)]
            vi = 0
            for i, (which, hc, lt) in enumerate(kq_tiles):
                emit_kq(hc, lt, which)
                want_v = ((i + 1) * NKC) // len(kq_tiles)
                while vi < want_v:
                    emit_v(vi)
                    vi += 1
            while vi < NKC:
                emit_v(vi)
                vi += 1

            # ---- attention
            # Software-pipelined emission: chunk kc's P@V matmuls are emitted
            # AFTER chunk kc+1's QK matmuls, so the PE always has independent
            # work in program order while the DVE mask-add + ACT exp of the
            # current chunk are still in flight. One mask DMA per ql tile
            # ([2048, 512] block) with 3 buffers -> 2-deep prefetch.
            mask_dma = nc.gpsimd if mask_ring == "gpsimd" else nc.scalar
            for qt in range(NQT):
                qsl = slice(qt * QT, (qt + 1) * QT)
                mk = mtp.tile([P, NKC, QT], MDT, name="mk", tag="mk")
                mask_dma.dma_start(
                    out=mk, in_=maskT[:, qsl].rearrange("(c p) n -> p c n", p=P))
                pvs = [
                    ps_pv.tile([P, HV], F32, name=f"pv{j}", tag=f"pv{j}")
                    for j in range(4)
                ]
                pending_pv = None  # (kc, p-tile) awaiting PV emission

                def emit_pv(kc, ptile):
                    for j in range(4):
                        nc.tensor.matmul(
                            pvs[j],
                            lhsT=ptile[:, j * P:(j + 1) * P],
                            rhs=v_r[:, kc, :],
                            start=(kc == 0), stop=(kc == NKC - 1),
                        )

                for kc in range(NKC):
                    st = ps_st.tile([P, QT], F32, name="st", tag="st")
                    for hc in range(2):
                        nc.tensor.matmul(
                            st,
                            lhsT=kT_r[:, hc, kc * P:(kc + 1) * P],
                            rhs=qT_r[:, hc, qsl],
                            start=(hc == 0), stop=(hc == 1),
                        )
                    if pending_pv is not None:
                        emit_pv(*pending_pv)
                    nc.vector.tensor_tensor(
                        out=st, in0=st, in1=mk[:, kc, :], op=mybir.AluOpType.add)
                    pt_ = ptp.tile([P, QT], F32R, name="pt", tag="pt")
                    nc.scalar.activation(
                        out=pt_, in_=st, func=mybir.ActivationFunctionType.Exp)
                    pending_pv = (kc, pt_)
                emit_pv(*pending_pv)

                # Ship the unnormalized partial [sum p*v | sum p]; the host
                # divides after combining the two key-halves. One DMA per
                # ql tile.
                ot = fin.tile([P, 4, HID + 1], F32, name="ot", tag="ot")
                for j in range(4):
                    nc.vector.tensor_copy(ot[:, j, :], pvs[j][:, :HID + 1])
                nc.sync.dma_start(
                    out=out[qt * QT:(qt + 1) * QT, :].rearrange(
                        "(j p) n -> p j n", p=P),
                    in_=ot,
                )
    nc.finalize()
    return nc


def _get_nc():
    key = "nc_turbo_" + "_".join(_mask_cfg())
    if key not in _CACHE:
        _CACHE[key] = _build()
    return _CACHE[key]


def kernel(embedding, mask, Wq, bq, Wk, bk, Wv, bv):
    embedding = np.asarray(embedding, dtype=np.float32)
    mask = np.asarray(mask, dtype=np.float32)
    Wq = np.asarray(Wq, dtype=np.float32)
    Wk = np.asarray(Wk, dtype=np.float32)
    Wv = np.asarray(Wv, dtype=np.float32)
    bq = np.asarray(bq, dtype=np.float32)
    bk = np.asarray(bk, dtype=np.float32)
    bv = np.asarray(bv, dtype=np.float32)

    _, mask_dt = _mask_cfg()
    MNP = F8NP if mask_dt == "f8" else ml_dtypes.bfloat16
    mscale = MASK_VAL if mask_dt == "f8" else np.float32(-100000.0)

    def pad_w(w, b, extra_one=False):
        wp = np.zeros((EPAD, HV if extra_one else HID), dtype=np.float32)
        wp[:EMB, :HID] = w
        wp[EMB, :HID] = b
        if extra_one:
            wp[EMB, HID] = 1.0
        return wp.astype(F16NP)

    wq_a = pad_w(Wq, bq)
    wk_a = pad_w(Wk, bk)
    wv_a = pad_w(Wv, bv, extra_one=True)

    in_maps = []
    for c in range(NCORES):
        b, half = divmod(c, 2)
        embT = np.zeros((EPAD, L), dtype=np.float32)
        embT[:EMB] = embedding[b].T
        embT[EMB] = 1.0
        embT16 = embT.astype(F16NP)
        ksl = slice(half * KL, (half + 1) * KL)
        mT = (mask[b].T[ksl, :] * mscale).astype(MNP)
        in_maps.append({
            "embT": embT16,
            "embTk": np.ascontiguousarray(embT16[:, ksl]),
            "wq": wq_a, "wk": wk_a, "wv": wv_a,
            "maskT": mT,
        })

    nc = _get_nc()
    trace = bool(int(os.environ.get("BASS_KERNEL_TRACE", "0")))
    res = run_bass_kernel_spmd(nc, in_maps, core_ids=list(range(NCORES)), trace=trace)
    _CACHE["last_results"] = res

    full = np.empty((B, L, HID), dtype=np.float32)
    for b in range(B):
        r0 = res.results[2 * b]["out"].astype(np.float64)
        r1 = res.results[2 * b + 1]["out"].astype(np.float64)
        num = r0[:, :HID] + r1[:, :HID]
        den = r0[:, HID:] + r1[:, HID:]
        full[b] = (num / den).astype(np.float32)
    return full


# revision 4
# speedup vs baseline: 1.2721x; 1.0055x over previous
"""Single-head attention (B=4, L=4096, EMB=312, HID=256) on 8 NeuronCores.

Sharding: data-parallel over batch (4) x key-parallel (2) = 8 cores. Each
core handles ALL 4096 queries against its half of the keys and returns the
UNNORMALIZED partial [sum_k p*v | sum_k p] rows; the host combines the two
halves as (o1+o2)/(s1+s2). Key-sharding (vs query-sharding) halves the
duplicated K/V projection work; only the Q projection is duplicated.

Device algorithm (per core):
  - Inputs arrive pre-transposed/padded from the host. emb and W* are fp16
    (values are small-range, so fp16's 11-bit mantissa beats bf16 and loads
    half the bytes of fp32); projections are single-pass fp16 matmuls
    accumulated in fp32 PSUM (1 cycle/row on the PE, same rate as bf16).
  - embT carries a ones-row at index EMB and W* carry the bias in that row,
    so projections fold the bias in. Wv has 2 extra columns: ones (gives the
    softmax row-sum through the P@V matmul) and zero padding (even N).
  - q/k/v are stored as fp32r; QK and PV run single-pass fp32r matmuls
    (1 cycle/row at these tile widths, ~tf32 operand precision).
  - Scores are computed transposed: sT[kl, ql] = kT-chunk^T @ qT, so the
    exp() output is directly the stationary operand for the P@V matmul —
    no on-device transposes anywhere.
  - Mask is host-side transposed and encoded as fp8e4m3 {0, -240}: adding
    -240 to a score makes exp() underflow to exactly 0.0 in fp32, which is
    indistinguishable from the reference's -1e5 (no row is fully masked).
    The DVE applies it additively to the score PSUM; exp() on ACT.
  - DMA discipline: the TimelineSim charges ~565-667ns of sequencer time
    per dma_start on the SP/ACT/DVE rings plus a shared-HWDGE hold, so
    transfers are batched: 12 emb block loads + 3 weight loads + 8 mask
    loads (one per query tile, on the gpsimd/SWDGE ring which bypasses
    HWDGE) + 8 output stores. ~31 DMAs total vs 232 in the bf16x2 version.
  - The raw partials (P@V columns + row-sum column) go back to the host,
    which normalizes after combining the key-halves.

Env overrides (debug): BASS_KERNEL_MASK_RING=gpsimd|scalar,
BASS_KERNEL_MASK_DT=f8|bf16.
"""
import os

import numpy as np
import ml_dtypes

import concourse.bacc as bacc
import concourse.tile as tile
from concourse import mybir, bass2jax
from concourse.bass_utils import run_bass_kernel_spmd

# Debug aid (opt-in): surface real compile errors from the PJRT compile
# hook, which the C++ bridge otherwise swallows.
if os.environ.get("BASS_KERNEL_DEBUG"):
    import functools as _ft
    import traceback as _tb
    _orig_hook = bass2jax.neuronx_cc_hook
    @_ft.wraps(_orig_hook)
    def _dbg_hook(*args, **kwargs):
        try:
            return _orig_hook(*args, **kwargs)
        except BaseException:
            _tb.print_exc()
            raise
    bass2jax.neuronx_cc_hook = _dbg_hook

EMB, HID, B, L = 312, 256, 4, 4096
NCORES = 8
P = 128
KL = L // 2            # key rows per core (key-parallel halves)
EPAD = 384             # emb dim padded to 3 partition chunks; row EMB is the ones-row
HV = HID + 2           # v columns: HID values | ones | zero pad (even N)
QT = 512               # ql tile width (PSUM bank = 512 fp32)
NKC = KL // P          # 16 kl chunks per core
NQT = L // QT          # 8 ql tiles per core (all queries)
NKT = KL // QT         # 4 l tiles for the k projection
MASK_VAL = np.float32(-240.0)   # exactly representable in fp8e4m3

F32 = mybir.dt.float32
F16 = mybir.dt.float16
F32R = mybir.dt.float32r
F8 = mybir.dt.float8e4
BF16 = mybir.dt.bfloat16
F16NP = np.float16
F8NP = ml_dtypes.float8_e4m3

_CACHE = {}


def _mask_cfg():
    ring = os.environ.get("BASS_KERNEL_MASK_RING", "gpsimd")
    dt = os.environ.get("BASS_KERNEL_MASK_DT", "f8")
    return ring, dt


def _build():
    mask_ring, mask_dt = _mask_cfg()
    MDT = F8 if mask_dt == "f8" else BF16

    nc = bacc.Bacc(None)

    embT = nc.dram_tensor("embT", [EPAD, L], F16, kind="ExternalInput")
    embTk = nc.dram_tensor("embTk", [EPAD, KL], F16, kind="ExternalInput")
    wq = nc.dram_tensor("wq", [EPAD, HID], F16, kind="ExternalInput")
    wk = nc.dram_tensor("wk", [EPAD, HID], F16, kind="ExternalInput")
    wv = nc.dram_tensor("wv", [EPAD, HV], F16, kind="ExternalInput")
    maskT = nc.dram_tensor("maskT", [KL, L], MDT, kind="ExternalInput")
    out = nc.dram_tensor("out", [L, HID + 1], F32, kind="ExternalOutput")

    with tile.TileContext(nc) as tc:
        with (
            tc.tile_pool(name="big", bufs=1) as big,
            tc.tile_pool(name="wp", bufs=1) as wp,
            tc.tile_pool(name="mt", bufs=3) as mtp,
            tc.tile_pool(name="pt", bufs=4) as ptp,
            tc.tile_pool(name="fin", bufs=2) as fin,
            tc.tile_pool(name="ps_st", bufs=4, space="PSUM") as ps_st,
            tc.tile_pool(name="ps_pv", bufs=1, space="PSUM") as ps_pv,
        ):
            # ---- input loads. Weight tensors ride the ACT ring; emb blocks
            # ride the SP ring, lowest columns first so the first projection
            # matmuls start a couple of us in. Each DMA covers all 3
            # emb-chunks of its column block (partition p reads rows
            # {p, 128+p, 256+p}).
            wk_t = wp.tile([P, 3, HID], F16, name="wk_t")
            wv_t = wp.tile([P, 3, HV], F16, name="wv_t")
            wq_t = wp.tile([P, 3, HID], F16, name="wq_t")
            for t, d in ((wk_t, wk), (wv_t, wv), (wq_t, wq)):
                nc.scalar.dma_start(
                    out=t, in_=d[:, :].rearrange("(c p) n -> p c n", p=P))

            embTk_t = big.tile([P, 3, KL], F16, name="embTk_t")
            embT_t = big.tile([P, 3, L], F16, name="embT_t")
            for t, d, n in ((embTk_t, embTk, KL), (embT_t, embT, L)):
                for b0 in range(0, n, QT):
                    nc.sync.dma_start(
                        out=t[:, :, b0:b0 + QT],
                        in_=d[:, b0:b0 + QT].rearrange("(c p) n -> p c n", p=P),
                    )

            kT_r = big.tile([P, 2, KL], F32R, name="kT_r")
            qT_r = big.tile([P, 2, L], F32R, name="qT_r")
            v_r = big.tile([P, NKC, HV], F32R, name="v_r")

            # ---- projections (single-pass fp16, fp32 PSUM accumulate).
            # q/k in [h(part), hc, l(free)] layout; v in [kl(part), klc, h].
            # k/q PSUM->SBUF copies go to the DVE and v copies to ACT so the
            # copy work never gates the PE during the projection phase.
            def emit_kq(hc, lt, which):
                ps = ps_st.tile([P, QT], F32, name="st", tag="st")
                w, e, dst = (
                    (wk_t, embTk_t, kT_r) if which == "k"
                    else (wq_t, embT_t, qT_r)
                )
                for ei in range(3):
                    nc.tensor.matmul(
                        ps,
                        lhsT=w[:, ei, hc * P:(hc + 1) * P],
                        rhs=e[:, ei, lt * QT:(lt + 1) * QT],
                        start=(ei == 0), stop=(ei == 2),
                    )
                nc.vector.tensor_copy(dst[:, hc, lt * QT:(lt + 1) * QT], ps)

            def emit_v(kc):
                ps = ps_st.tile([P, QT], F32, name="st", tag="st")
                for ei in range(3):
                    nc.tensor.matmul(
                        ps[:, :HV],
                        lhsT=embTk_t[:, ei, kc * P:(kc + 1) * P],
                        rhs=wv_t[:, ei, :],
                        start=(ei == 0), stop=(ei == 2),
                    )
                nc.scalar.copy(out=v_r[:, kc, :], in_=ps[:, :HV])

            kq_tiles = [("k", hc, lt) for lt in range(NKT) for hc in range(2)]
            kq_tiles += [("q", hc, lt) for lt in range(NQT) for hc in range(2)]
            vi = 0
            for i, (which, hc, lt) in enumerate(kq_tiles):
                emit_kq(hc, lt, which)
                want_v = ((i + 1) * NKC) // len(kq_tiles)
                while vi < want_v:
                    emit_v(vi)
                    vi += 1
            while vi < NKC:
                emit_v(vi)
                vi += 1

            # ---- attention
            # Uniform lag-2 software pipeline carried ACROSS ql-tile
            # boundaries: chunk kc's P@V matmuls are emitted after chunk
            # kc+2's QK matmuls (even across ql tiles), so the PE always has
            # ~2 tiles of independent work in program order while the DVE
            # mask-add + ACT exp + pv-bank WAR release of the current chunk
            # are still in flight. One mask DMA per ql tile ([2048, 512]
            # block, gpsimd/SWDGE ring) with 3 buffers -> 2-deep prefetch.
            # pv PSUM banks are reused every tile; the staging copies (DVE)
            # are emitted at the kc==15 flush, which under lag-2 lands
            # between the next tile's mask-adds early enough that the new
            # accumulation's per-bank WAR is satisfied before the PE gets
            # there. pvs allocation for a tile happens at its kc==0 flush,
            # after those copies.
            mask_dma = nc.gpsimd if mask_ring == "gpsimd" else nc.scalar
            from collections import deque

            pvs_box = [None]

            def emit_pv(oqt, kc, ptile):
                if kc == 0:
                    pvs_box[0] = [
                        ps_pv.tile([P, HV], F32, name=f"pv{j}", tag=f"pv{j}")
                        for j in range(4)
                    ]
                pvs = pvs_box[0]
                for j in range(4):
                    nc.tensor.matmul(
                        pvs[j],
                        lhsT=ptile[:, j * P:(j + 1) * P],
                        rhs=v_r[:, kc, :],
                        start=(kc == 0), stop=(kc == NKC - 1),
                    )
                if kc == NKC - 1:
                    # Ship the unnormalized partial [sum p*v | sum p]; the
                    # host divides after combining the two key-halves.
                    ot = fin.tile([P, 4, HID + 1], F32, name="ot", tag="ot")
                    for j in range(4):
                        nc.vector.tensor_copy(ot[:, j, :], pvs[j][:, :HID + 1])
                    nc.sync.dma_start(
                        out=out[oqt * QT:(oqt + 1) * QT, :].rearrange(
                            "(j p) n -> p j n", p=P),
                        in_=ot,
                    )

            pending = deque()  # (qt, kc, p-tile) awaiting PV emission
            for qt in range(NQT):
                qsl = slice(qt * QT, (qt + 1) * QT)
                mk = mtp.tile([P, NKC, QT], MDT, name="mk", tag="mk")
                mask_dma.dma_start(
                    out=mk, in_=maskT[:, qsl].rearrange("(c p) n -> p c n", p=P))
                for kc in range(NKC):
                    st = ps_st.tile([P, QT], F32, name="st", tag="st")
                    for hc in range(2):
                        nc.tensor.matmul(
                            st,
                            lhsT=kT_r[:, hc, kc * P:(kc + 1) * P],
                            rhs=qT_r[:, hc, qsl],
                            start=(hc == 0), stop=(hc == 1),
                        )
                    if len(pending) == 2:
                        emit_pv(*pending.popleft())
                    nc.vector.tensor_tensor(
                        out=st, in0=st, in1=mk[:, kc, :], op=mybir.AluOpType.add)
                    pt_ = ptp.tile([P, QT], F32R, name="pt", tag="pt")
                    nc.scalar.activation(
                        out=pt_, in_=st, func=mybir.ActivationFunctionType.Exp)
                    pending.append((qt, kc, pt_))
            while pending:
                emit_pv(*pending.popleft())
    nc.finalize()
    return nc


def _get_nc():
    key = "nc_turbo_" + "_".join(_mask_cfg())
    if key not in _CACHE:
        _CACHE[key] = _build()
    return _CACHE[key]


def kernel(embedding, mask, Wq, bq, Wk, bk, Wv, bv):
    embedding = np.asarray(embedding, dtype=np.float32)
    mask = np.asarray(mask, dtype=np.float32)
    Wq = np.asarray(Wq, dtype=np.float32)
    Wk = np.asarray(Wk, dtype=np.float32)
    Wv = np.asarray(Wv, dtype=np.float32)
    bq = np.asarray(bq, dtype=np.float32)
    bk = np.asarray(bk, dtype=np.float32)
    bv = np.asarray(bv, dtype=np.float32)

    _, mask_dt = _mask_cfg()
    MNP = F8NP if mask_dt == "f8" else ml_dtypes.bfloat16
    mscale = MASK_VAL if mask_dt == "f8" else np.float32(-100000.0)

    def pad_w(w, b, extra_one=False):
        wp = np.zeros((EPAD, HV if extra_one else HID), dtype=np.float32)
        wp[:EMB, :HID] = w
        wp[EMB, :HID] = b
        if extra_one:
            wp[EMB, HID] = 1.0
        return wp.astype(F16NP)

    wq_a = pad_w(Wq, bq)
    wk_a = pad_w(Wk, bk)
    wv_a = pad_w(Wv, bv, extra_one=True)

    in_maps = []
    for c in range(NCORES):
        b, half = divmod(c, 2)
        embT = np.zeros((EPAD, L), dtype=np.float32)
        embT[:EMB] = embedding[b].T
        embT[EMB] = 1.0
        embT16 = embT.astype(F16NP)
        ksl = slice(half * KL, (half + 1) * KL)
        mT = (mask[b].T[ksl, :] * mscale).astype(MNP)
        in_maps.append({
            "embT": embT16,
            "embTk": np.ascontiguousarray(embT16[:, ksl]),
            "wq": wq_a, "wk": wk_a, "wv": wv_a,
            "maskT": mT,
        })

    nc = _get_nc()
    trace = bool(int(os.environ.get("BASS_KERNEL_TRACE", "0")))
    res = run_bass_kernel_spmd(nc, in_maps, core_ids=list(range(NCORES)), trace=trace)
    _CACHE["last_results"] = res

    full = np.empty((B, L, HID), dtype=np.float32)
    for b in range(B):
        r0 = res.results[2 * b]["out"].astype(np.float64)
        r1 = res.results[2 * b + 1]["out"].astype(np.float64)
        num = r0[:, :HID] + r1[:, :HID]
        den = r0[:, HID:] + r1[:, HID:]
        full[b] = (num / den).astype(np.float32)
    return full


# revision 11
# speedup vs baseline: 1.3737x; 1.0799x over previous
"""Single-head attention (B=4, L=4096, EMB=312, HID=256) on 8 NeuronCores.

Sharding: data-parallel over batch (4) x key-parallel (2) = 8 cores. Each
core handles ALL 4096 queries against its half of the keys and returns the
UNNORMALIZED partial [sum_k p*v | sum_k p] rows; the host combines the two
halves as (o1+o2)/(s1+s2). Key-sharding (vs query-sharding) halves the
duplicated K/V projection work; only the Q projection is duplicated.

Device algorithm (per core):
  - Inputs arrive pre-transposed/padded from the host. emb and W* are fp16
    (values are small-range, so fp16's 11-bit mantissa beats bf16 and loads
    half the bytes of fp32); projections are single-pass fp16 matmuls
    accumulated in fp32 PSUM (1 cycle/row on the PE, same rate as bf16).
  - embT carries a ones-row at index EMB and W* carry the bias in that row,
    so projections fold the bias in. Wv has 2 extra columns: ones (gives the
    softmax row-sum through the P@V matmul) and zero padding (even N).
  - q/k/v are stored as fp32r; QK and PV run single-pass fp32r matmuls
    (1 cycle/row at these tile widths, ~tf32 operand precision).
  - Scores are computed transposed: sT[kl, ql] = kT-chunk^T @ qT, so the
    exp() output is directly the stationary operand for the P@V matmul —
    no on-device transposes anywhere.
  - Mask is host-side transposed and encoded as fp8e4m3 {0, -240}: adding
    -240 to a score makes exp() underflow to exactly 0.0 in fp32, which is
    indistinguishable from the reference's -1e5 (no row is fully masked).
    The DVE applies it additively to the score PSUM; exp() on ACT.
  - DMA discipline: the TimelineSim charges ~565-667ns of sequencer time
    per dma_start on the SP/ACT/DVE rings plus a shared-HWDGE hold, so
    transfers are batched: 12 emb block loads + 3 weight loads + 8 mask
    loads (one per query tile, on the gpsimd/SWDGE ring which bypasses
    HWDGE) + 8 output stores. ~31 DMAs total vs 232 in the bf16x2 version.
  - The raw partials (P@V columns + row-sum column) go back to the host,
    which normalizes after combining the key-halves.

Env overrides (debug): BASS_KERNEL_MASK_RING=gpsimd|scalar,
BASS_KERNEL_MASK_DT=f8|bf16.
"""
import os

import numpy as np
import ml_dtypes

import concourse.bacc as bacc
import concourse.tile as tile
from concourse import mybir, bass2jax
from concourse.bass_utils import run_bass_kernel_spmd

# Debug aid (opt-in): surface real compile errors from the PJRT compile
# hook, which the C++ bridge otherwise swallows.
if os.environ.get("BASS_KERNEL_DEBUG"):
    import functools as _ft
    import traceback as _tb
    _orig_hook = bass2jax.neuronx_cc_hook
    @_ft.wraps(_orig_hook)
    def _dbg_hook(*args, **kwargs):
        try:
            return _orig_hook(*args, **kwargs)
        except BaseException:
            _tb.print_exc()
            raise
    bass2jax.neuronx_cc_hook = _dbg_hook

EMB, HID, B, L = 312, 256, 4, 4096
NCORES = 8
P = 128
KL = L // 2            # key rows per core (key-parallel halves)
EPAD = 384             # emb dim padded to 3 partition chunks; row EMB is the ones-row
HV = HID + 2           # v columns: HID values | ones | zero pad (even N)
QT = 512               # ql tile width (PSUM bank = 512 fp32)
NKC = KL // P          # 16 kl chunks per core
NQT = L // QT          # 8 ql tiles per core (all queries)
NKT = KL // QT         # 4 l tiles for the k projection
MASK_VAL = np.float32(-240.0)   # exactly representable in fp8e4m3

F32 = mybir.dt.float32
F16 = mybir.dt.float16
F32R = mybir.dt.float32r
F8 = mybir.dt.float8e4
BF16 = mybir.dt.bfloat16
F16NP = np.float16
F8NP = ml_dtypes.float8_e4m3

_CACHE = {}


def _mask_cfg():
    ring = os.environ.get("BASS_KERNEL_MASK_RING", "gpsimd")
    dt = os.environ.get("BASS_KERNEL_MASK_DT", "f8")
    return ring, dt


def _build():
    mask_ring, mask_dt = _mask_cfg()
    MDT = F8 if mask_dt == "f8" else BF16

    nc = bacc.Bacc(None)

    embT = nc.dram_tensor("embT", [EPAD, L], F16, kind="ExternalInput")
    wq = nc.dram_tensor("wq", [EPAD, HID], F16, kind="ExternalInput")
    wk = nc.dram_tensor("wk", [EPAD, HID], F16, kind="ExternalInput")
    wv = nc.dram_tensor("wv", [EPAD, HV], F16, kind="ExternalInput")
    maskT = nc.dram_tensor("maskT", [KL, L], MDT, kind="ExternalInput")
    out = nc.dram_tensor("out", [L, HID + 1], F32, kind="ExternalOutput")

    with tile.TileContext(nc) as tc:
        with (
            tc.tile_pool(name="big", bufs=1) as big,
            tc.tile_pool(name="wp", bufs=1) as wp,
            tc.tile_pool(name="mt", bufs=2) as mtp,
            tc.tile_pool(name="pt", bufs=6) as ptp,
            tc.tile_pool(name="fin", bufs=2) as fin,
            tc.tile_pool(name="ps_st", bufs=4, space="PSUM") as ps_st,
            tc.tile_pool(name="ps_pv", bufs=1, space="PSUM") as ps_pv,
        ):
            # ---- input loads. Weight tensors ride the ACT ring; emb blocks
            # ride the SP ring, lowest columns first so the first projection
            # matmuls start a couple of us in. Each DMA covers all 3
            # emb-chunks of its column block (partition p reads rows
            # {p, 128+p, 256+p}).
            wk_t = wp.tile([P, 3, HID], F16, name="wk_t")
            wv_t = wp.tile([P, 3, HV], F16, name="wv_t")
            wq_t = wp.tile([P, 3, HID], F16, name="wq_t")
            for t, d in ((wk_t, wk), (wv_t, wv), (wq_t, wq)):
                nc.scalar.dma_start(
                    out=t, in_=d[:, :].rearrange("(c p) n -> p c n", p=P))

            # The host rotates each core's query columns so its key-half
            # occupies columns 0..KL-1 (undone host-side on the output), so
            # the K/V projections read a PREFIX of embT and no separate
            # embTk load is needed.
            embT_t = big.tile([P, 3, L], F16, name="embT_t")
            for b0 in range(0, L, QT):
                nc.sync.dma_start(
                    out=embT_t[:, :, b0:b0 + QT],
                    in_=embT[:, b0:b0 + QT].rearrange("(c p) n -> p c n", p=P),
                )

            kT_r = big.tile([P, 2, KL], F32R, name="kT_r")
            qT_r = big.tile([P, 2, L], F32R, name="qT_r")
            v_r = big.tile([P, NKC, HV], F32R, name="v_r")

            # ---- projections (single-pass fp16, fp32 PSUM accumulate).
            # q/k in [h(part), hc, l(free)] layout; v in [kl(part), klc, h].
            # k/q PSUM->SBUF copies go to the DVE and v copies to ACT so the
            # copy work never gates the PE during the projection phase.
            def emit_kq(hc, lt, which):
                ps = ps_st.tile([P, QT], F32, name="st", tag="st")
                w, dst = (wk_t, kT_r) if which == "k" else (wq_t, qT_r)
                for ei in range(3):
                    nc.tensor.matmul(
                        ps,
                        lhsT=w[:, ei, hc * P:(hc + 1) * P],
                        rhs=embT_t[:, ei, lt * QT:(lt + 1) * QT],
                        start=(ei == 0), stop=(ei == 2),
                    )
                nc.vector.tensor_copy(dst[:, hc, lt * QT:(lt + 1) * QT], ps)

            def emit_v(kc):
                ps = ps_st.tile([P, QT], F32, name="st", tag="st")
                for ei in range(3):
                    nc.tensor.matmul(
                        ps[:, :HV],
                        lhsT=embT_t[:, ei, kc * P:(kc + 1) * P],
                        rhs=wv_t[:, ei, :],
                        start=(ei == 0), stop=(ei == 2),
                    )
                nc.scalar.copy(out=v_r[:, kc, :], in_=ps[:, :HV])

            kq_tiles = [("k", hc, lt) for lt in range(NKT) for hc in range(2)]
            kq_tiles += [("q", hc, lt) for lt in range(NQT) for hc in range(2)]
            vi = 0
            for i, (which, hc, lt) in enumerate(kq_tiles):
                emit_kq(hc, lt, which)
                want_v = ((i + 1) * NKC) // len(kq_tiles)
                while vi < want_v:
                    emit_v(vi)
                    vi += 1
            while vi < NKC:
                emit_v(vi)
                vi += 1

            # ---- attention
            # Uniform lag-2 software pipeline carried ACROSS ql-tile
            # boundaries: chunk kc's P@V matmuls are emitted after chunk
            # kc+2's QK matmuls (even across ql tiles), so the PE always has
            # ~2 tiles of independent work in program order while the DVE
            # mask-add + ACT exp + pv-bank WAR release of the current chunk
            # are still in flight. One mask DMA per ql tile ([2048, 512]
            # block, gpsimd/SWDGE ring) with 3 buffers -> 2-deep prefetch.
            # pv PSUM banks are reused every tile; the staging copies (DVE)
            # are emitted at the kc==15 flush, which under lag-2 lands
            # between the next tile's mask-adds early enough that the new
            # accumulation's per-bank WAR is satisfied before the PE gets
            # there. pvs allocation for a tile happens at its kc==0 flush,
            # after those copies.
            mask_dma = nc.gpsimd if mask_ring == "gpsimd" else nc.scalar
            from collections import deque

            # pv accumulators: ONE PSUM tile [P, 4, 512] so each j block
            # owns exactly one 2KB bank (matmul outputs stay bank-local) and
            # the output staging is a single DVE copy instead of four.
            pvs_box = [None]
            LAG = 3

            def emit_pv(oqt, kc, ptile):
                if kc == 0:
                    pvs_box[0] = ps_pv.tile([P, 4, QT], F32, name="pv", tag="pv")
                pv = pvs_box[0]
                for j in range(4):
                    nc.tensor.matmul(
                        pv[:, j, :HV],
                        lhsT=ptile[:, j * P:(j + 1) * P],
                        rhs=v_r[:, kc, :],
                        start=(kc == 0), stop=(kc == NKC - 1),
                    )
                if kc == NKC - 1:
                    # Ship the unnormalized partial [sum p*v | sum p]; the
                    # host divides after combining the two key-halves.
                    ot = fin.tile([P, 4, HID + 1], F32, name="ot", tag="ot")
                    nc.vector.tensor_copy(ot, pv[:, :, :HID + 1])
                    nc.sync.dma_start(
                        out=out[oqt * QT:(oqt + 1) * QT, :].rearrange(
                            "(j p) n -> p j n", p=P),
                        in_=ot,
                    )

            pending = deque()  # (qt, kc, p-tile) awaiting PV emission
            for qt in range(NQT):
                qsl = slice(qt * QT, (qt + 1) * QT)
                mk = mtp.tile([P, NKC, QT], MDT, name="mk", tag="mk")
                mask_dma.dma_start(
                    out=mk, in_=maskT[:, qsl].rearrange("(c p) n -> p c n", p=P))
                for kc in range(NKC):
                    st = ps_st.tile([P, QT], F32, name="st", tag="st")
                    for hc in range(2):
                        nc.tensor.matmul(
                            st,
                            lhsT=kT_r[:, hc, kc * P:(kc + 1) * P],
                            rhs=qT_r[:, hc, qsl],
                            start=(hc == 0), stop=(hc == 1),
                        )
                    if len(pending) == LAG:
                        emit_pv(*pending.popleft())
                    nc.vector.tensor_tensor(
                        out=st, in0=st, in1=mk[:, kc, :], op=mybir.AluOpType.add)
                    pt_ = ptp.tile([P, QT], F32R, name="pt", tag="pt")
                    nc.scalar.activation(
                        out=pt_, in_=st, func=mybir.ActivationFunctionType.Exp)
                    pending.append((qt, kc, pt_))
            while pending:
                emit_pv(*pending.popleft())
    nc.finalize()
    return nc


def _get_nc():
    key = "nc_turbo_" + "_".join(_mask_cfg())
    if key not in _CACHE:
        _CACHE[key] = _build()
    return _CACHE[key]


def kernel(embedding, mask, Wq, bq, Wk, bk, Wv, bv):
    embedding = np.asarray(embedding, dtype=np.float32)
    mask = np.asarray(mask, dtype=np.float32)
    Wq = np.asarray(Wq, dtype=np.float32)
    Wk = np.asarray(Wk, dtype=np.float32)
    Wv = np.asarray(Wv, dtype=np.float32)
    bq = np.asarray(bq, dtype=np.float32)
    bk = np.asarray(bk, dtype=np.float32)
    bv = np.asarray(bv, dtype=np.float32)

    _, mask_dt = _mask_cfg()
    MNP = F8NP if mask_dt == "f8" else ml_dtypes.bfloat16
    mscale = MASK_VAL if mask_dt == "f8" else np.float32(-100000.0)

    def pad_w(w, b, extra_one=False):
        wp = np.zeros((EPAD, HV if extra_one else HID), dtype=np.float32)
        wp[:EMB, :HID] = w
        wp[EMB, :HID] = b
        if extra_one:
            wp[EMB, HID] = 1.0
        return wp.astype(F16NP)

    wq_a = pad_w(Wq, bq)
    wk_a = pad_w(Wk, bk)
    wv_a = pad_w(Wv, bv, extra_one=True)

    # Each core's query columns are rotated so its key-half occupies
    # columns 0..KL-1: the device then projects K/V from a prefix of the
    # same embT tile (no separate embTk load) and the host un-rotates the
    # output rows after the gather. half=0 is the identity; half=1 swaps
    # the two halves (an involution).
    in_maps = []
    for c in range(NCORES):
        b, half = divmod(c, 2)
        embT = np.zeros((EPAD, L), dtype=np.float32)
        embT[:EMB] = embedding[b].T
        embT[EMB] = 1.0
        embT16 = embT.astype(F16NP)
        ksl = slice(half * KL, (half + 1) * KL)
        mT = (mask[b].T[ksl, :] * mscale).astype(MNP)
        if half == 1:
            embT16 = np.ascontiguousarray(
                np.concatenate([embT16[:, KL:], embT16[:, :KL]], axis=1))
            mT = np.ascontiguousarray(
                np.concatenate([mT[:, KL:], mT[:, :KL]], axis=1))
        in_maps.append({
            "embT": embT16,
            "wq": wq_a, "wk": wk_a, "wv": wv_a,
            "maskT": mT,
        })

    nc = _get_nc()
    trace = bool(int(os.environ.get("BASS_KERNEL_TRACE", "0")))
    res = run_bass_kernel_spmd(nc, in_maps, core_ids=list(range(NCORES)), trace=trace)
    _CACHE["last_results"] = res

    full = np.empty((B, L, HID), dtype=np.float32)
    for b in range(B):
        r0 = res.results[2 * b]["out"].astype(np.float64)
        r1 = res.results[2 * b + 1]["out"].astype(np.float64)
        r1 = np.concatenate([r1[KL:], r1[:KL]], axis=0)  # un-rotate half=1
        num = r0[:, :HID] + r1[:, :HID]
        den = r0[:, HID:] + r1[:, HID:]
        full[b] = (num / den).astype(np.float32)
    return full


# revision 14
# speedup vs baseline: 1.3940x; 1.0148x over previous
"""Single-head attention (B=4, L=4096, EMB=312, HID=256) on 8 NeuronCores.

Sharding: data-parallel over batch (4) x key-parallel (2) = 8 cores. Each
core handles ALL 4096 queries against its half of the keys and returns the
UNNORMALIZED partial [sum_k p*v | sum_k p] rows; the host combines the two
halves as (o1+o2)/(s1+s2). Key-sharding (vs query-sharding) halves the
duplicated K/V projection work; only the Q projection is duplicated.

Device algorithm (per core):
  - Inputs arrive pre-transposed/padded from the host. emb and W* are fp16
    (values are small-range, so fp16's 11-bit mantissa beats bf16 and loads
    half the bytes of fp32); projections are single-pass fp16 matmuls
    accumulated in fp32 PSUM (1 cycle/row on the PE, same rate as bf16).
  - embT carries a ones-row at index EMB and W* carry the bias in that row,
    so projections fold the bias in. Wv has 2 extra columns: ones (gives the
    softmax row-sum through the P@V matmul) and zero padding (even N).
  - q/k/v are stored as fp32r; QK and PV run single-pass fp32r matmuls
    (1 cycle/row at these tile widths, ~tf32 operand precision).
  - Scores are computed transposed: sT[kl, ql] = kT-chunk^T @ qT, so the
    exp() output is directly the stationary operand for the P@V matmul —
    no on-device transposes anywhere.
  - Mask is host-side transposed and encoded as fp8e4m3 {0, -240}: adding
    -240 to a score makes exp() underflow to exactly 0.0 in fp32, which is
    indistinguishable from the reference's -1e5 (no row is fully masked).
    The DVE applies it additively to the score PSUM; exp() on ACT.
  - DMA discipline: the TimelineSim charges ~565-667ns of sequencer time
    per dma_start on the SP/ACT/DVE rings plus a shared-HWDGE hold, so
    transfers are batched: 12 emb block loads + 3 weight loads + 8 mask
    loads (one per query tile, on the gpsimd/SWDGE ring which bypasses
    HWDGE) + 8 output stores. ~31 DMAs total vs 232 in the bf16x2 version.
  - The raw partials (P@V columns + row-sum column) go back to the host,
    which normalizes after combining the key-halves.

Env overrides (debug): BASS_KERNEL_MASK_RING=gpsimd|scalar,
BASS_KERNEL_MASK_DT=f8|bf16.
"""
import os

import numpy as np
import ml_dtypes

import concourse.bacc as bacc
import concourse.tile as tile
from concourse import mybir, bass2jax
from concourse.bass_utils import run_bass_kernel_spmd

# Debug aid (opt-in): surface real compile errors from the PJRT compile
# hook, which the C++ bridge otherwise swallows.
if os.environ.get("BASS_KERNEL_DEBUG"):
    import functools as _ft
    import traceback as _tb
    _orig_hook = bass2jax.neuronx_cc_hook
    @_ft.wraps(_orig_hook)
    def _dbg_hook(*args, **kwargs):
        try:
            return _orig_hook(*args, **kwargs)
        except BaseException:
            _tb.print_exc()
            raise
    bass2jax.neuronx_cc_hook = _dbg_hook

EMB, HID, B, L = 312, 256, 4, 4096
NCORES = 8
P = 128
KL = L // 2            # key rows per core (key-parallel halves)
EPAD = 384             # emb dim padded to 3 partition chunks; row EMB is the ones-row
HV = HID + 2           # v columns: HID values | ones | zero pad (even N)
QT = 512               # ql tile width (PSUM bank = 512 fp32)
NKC = KL // P          # 16 kl chunks per core
NQT = L // QT          # 8 ql tiles per core (all queries)
NKT = KL // QT         # 4 l tiles for the k projection
MASK_VAL = np.float32(-240.0)   # exactly representable in fp8e4m3

F32 = mybir.dt.float32
F16 = mybir.dt.float16
F32R = mybir.dt.float32r
F8 = mybir.dt.float8e4
BF16 = mybir.dt.bfloat16
F16NP = np.float16
F8NP = ml_dtypes.float8_e4m3

_CACHE = {}


def _mask_cfg():
    ring = os.environ.get("BASS_KERNEL_MASK_RING", "gpsimd")
    dt = os.environ.get("BASS_KERNEL_MASK_DT", "f8")
    return ring, dt


def _build():
    mask_ring, mask_dt = _mask_cfg()
    MDT = F8 if mask_dt == "f8" else BF16

    nc = bacc.Bacc(None)

    embT = nc.dram_tensor("embT", [EPAD, L], F16, kind="ExternalInput")
    wq = nc.dram_tensor("wq", [EPAD, HID], F16, kind="ExternalInput")
    wk = nc.dram_tensor("wk", [EPAD, HID], F16, kind="ExternalInput")
    wv = nc.dram_tensor("wv", [EPAD, HV], F16, kind="ExternalInput")
    maskT = nc.dram_tensor("maskT", [KL, L], MDT, kind="ExternalInput")
    out = nc.dram_tensor("out", [L, HID + 1], F32, kind="ExternalOutput")

    with tile.TileContext(nc) as tc:
        with (
            tc.tile_pool(name="big", bufs=1) as big,
            tc.tile_pool(name="wp", bufs=1) as wp,
            tc.tile_pool(name="mt", bufs=2) as mtp,
            tc.tile_pool(name="pt", bufs=6) as ptp,
            tc.tile_pool(name="fin", bufs=2) as fin,
            tc.tile_pool(name="ps_st", bufs=4, space="PSUM") as ps_st,
            tc.tile_pool(name="ps_pv", bufs=1, space="PSUM") as ps_pv,
        ):
            # ---- input loads. Weight tensors ride the ACT ring; emb blocks
            # ride the SP ring, lowest columns first so the first projection
            # matmuls start a couple of us in. Each DMA covers all 3
            # emb-chunks of its column block (partition p reads rows
            # {p, 128+p, 256+p}).
            wk_t = wp.tile([P, 3, HID], F16, name="wk_t")
            wv_t = wp.tile([P, 3, HV], F16, name="wv_t")
            wq_t = wp.tile([P, 3, HID], F16, name="wq_t")
            for t, d in ((wk_t, wk), (wv_t, wv), (wq_t, wq)):
                nc.scalar.dma_start(
                    out=t, in_=d[:, :].rearrange("(c p) n -> p c n", p=P))

            # The host rotates each core's query columns so its key-half
            # occupies columns 0..KL-1 (undone host-side on the output), so
            # the K/V projections read a PREFIX of embT and no separate
            # embTk load is needed.
            embT_t = big.tile([P, 3, L], F16, name="embT_t")
            for b0 in range(0, L, QT):
                if b0 == 0:
                    # Split the first block per emb-chunk so the very first
                    # projection matmul (which only needs chunk 0) starts
                    # ~1us earlier.
                    for cch in range(3):
                        nc.sync.dma_start(
                            out=embT_t[:, cch, 0:QT],
                            in_=embT[cch * P:(cch + 1) * P, 0:QT],
                        )
                else:
                    nc.sync.dma_start(
                        out=embT_t[:, :, b0:b0 + QT],
                        in_=embT[:, b0:b0 + QT].rearrange("(c p) n -> p c n", p=P),
                    )

            kT_r = big.tile([P, 2, KL], F32R, name="kT_r")
            qT_r = big.tile([P, 2, L], F32R, name="qT_r")
            v_r = big.tile([P, NKC, HV], F32R, name="v_r")

            # ---- projections (single-pass fp16, fp32 PSUM accumulate).
            # q/k in [h(part), hc, l(free)] layout; v in [kl(part), klc, h].
            # k/q PSUM->SBUF copies go to the DVE and v copies to ACT so the
            # copy work never gates the PE during the projection phase.
            def emit_kq(hc, lt, which):
                ps = ps_st.tile([P, QT], F32, name="st", tag="st")
                w, dst = (wk_t, kT_r) if which == "k" else (wq_t, qT_r)
                for ei in range(3):
                    nc.tensor.matmul(
                        ps,
                        lhsT=w[:, ei, hc * P:(hc + 1) * P],
                        rhs=embT_t[:, ei, lt * QT:(lt + 1) * QT],
                        start=(ei == 0), stop=(ei == 2),
                    )
                nc.vector.tensor_copy(dst[:, hc, lt * QT:(lt + 1) * QT], ps)

            def emit_v(kc):
                ps = ps_st.tile([P, QT], F32, name="st", tag="st")
                for ei in range(3):
                    nc.tensor.matmul(
                        ps[:, :HV],
                        lhsT=embT_t[:, ei, kc * P:(kc + 1) * P],
                        rhs=wv_t[:, ei, :],
                        start=(ei == 0), stop=(ei == 2),
                    )
                nc.scalar.copy(out=v_r[:, kc, :], in_=ps[:, :HV])

            kq_tiles = [("k", hc, lt) for lt in range(NKT) for hc in range(2)]
            kq_tiles += [("q", hc, lt) for lt in range(NQT) for hc in range(2)]
            vi = 0
            for i, (which, hc, lt) in enumerate(kq_tiles):
                emit_kq(hc, lt, which)
                want_v = ((i + 1) * NKC) // len(kq_tiles)
                while vi < want_v:
                    emit_v(vi)
                    vi += 1
            while vi < NKC:
                emit_v(vi)
                vi += 1

            # ---- attention
            # Uniform lag-2 software pipeline carried ACROSS ql-tile
            # boundaries: chunk kc's P@V matmuls are emitted after chunk
            # kc+2's QK matmuls (even across ql tiles), so the PE always has
            # ~2 tiles of independent work in program order while the DVE
            # mask-add + ACT exp + pv-bank WAR release of the current chunk
            # are still in flight. One mask DMA per ql tile ([2048, 512]
            # block, gpsimd/SWDGE ring) with 3 buffers -> 2-deep prefetch.
            # pv PSUM banks are reused every tile; the staging copies (DVE)
            # are emitted at the kc==15 flush, which under lag-2 lands
            # between the next tile's mask-adds early enough that the new
            # accumulation's per-bank WAR is satisfied before the PE gets
            # there. pvs allocation for a tile happens at its kc==0 flush,
            # after those copies.
            mask_dma = nc.gpsimd if mask_ring == "gpsimd" else nc.scalar
            from collections import deque

            # pv accumulators: ONE PSUM tile [P, 4, 512] so each j block
            # owns exactly one 2KB bank (matmul outputs stay bank-local) and
            # the output staging is a single DVE copy instead of four.
            pvs_box = [None]
            LAG = 3

            def emit_pv(oqt, kc, ptile):
                if kc == 0:
                    pvs_box[0] = ps_pv.tile([P, 4, QT], F32, name="pv", tag="pv")
                pv = pvs_box[0]
                for j in range(4):
                    nc.tensor.matmul(
                        pv[:, j, :HV],
                        lhsT=ptile[:, j * P:(j + 1) * P],
                        rhs=v_r[:, kc, :],
                        start=(kc == 0), stop=(kc == NKC - 1),
                    )
                if kc == NKC - 1:
                    # Ship the unnormalized partial [sum p*v | sum p]; the
                    # host divides after combining the two key-halves. For
                    # the last ql tile the copies+stores go per-j so the
                    # kernel tail isn't serialized behind one fused copy.
                    if oqt == NQT - 1:
                        for j in range(4):
                            otj = fin.tile([P, HID + 1], F32, name="otj",
                                           tag=f"otj{j}")
                            nc.vector.tensor_copy(otj, pv[:, j, :HID + 1])
                            r0 = (oqt * 4 + j) * P
                            nc.sync.dma_start(out=out[r0:r0 + P, :], in_=otj)
                    else:
                        ot = fin.tile([P, 4, HID + 1], F32, name="ot", tag="ot")
                        nc.vector.tensor_copy(ot, pv[:, :, :HID + 1])
                        nc.sync.dma_start(
                            out=out[oqt * QT:(oqt + 1) * QT, :].rearrange(
                                "(j p) n -> p j n", p=P),
                            in_=ot,
                        )

            pending = deque()  # (qt, kc, p-tile) awaiting PV emission
            for qt in range(NQT):
                qsl = slice(qt * QT, (qt + 1) * QT)
                mk = mtp.tile([P, NKC, QT], MDT, name="mk", tag="mk")
                # The first two mask loads ride the SP ring, whose in-order
                # program puts them AFTER the embT blocks — otherwise the
                # Pool ring issues them at t=0 and their transfers preempt
                # the startup emb loads on the shared DMA engines. Later
                # tiles (gated by the 2-buffer pool anyway) use the Pool
                # ring, keeping the SP ring free for output stores.
                ring = nc.sync if qt < 2 else mask_dma
                ring.dma_start(
                    out=mk, in_=maskT[:, qsl].rearrange("(c p) n -> p c n", p=P))
                for kc in range(NKC):
                    st = ps_st.tile([P, QT], F32, name="st", tag="st")
                    for hc in range(2):
                        nc.tensor.matmul(
                            st,
                            lhsT=kT_r[:, hc, kc * P:(kc + 1) * P],
                            rhs=qT_r[:, hc, qsl],
                            start=(hc == 0), stop=(hc == 1),
                        )
                    if len(pending) == LAG:
                        emit_pv(*pending.popleft())
                    nc.vector.tensor_tensor(
                        out=st, in0=st, in1=mk[:, kc, :], op=mybir.AluOpType.add)
                    pt_ = ptp.tile([P, QT], F32R, name="pt", tag="pt")
                    nc.scalar.activation(
                        out=pt_, in_=st, func=mybir.ActivationFunctionType.Exp)
                    pending.append((qt, kc, pt_))
            while pending:
                emit_pv(*pending.popleft())
    nc.finalize()
    return nc


def _get_nc():
    key = "nc_turbo_" + "_".join(_mask_cfg())
    if key not in _CACHE:
        _CACHE[key] = _build()
    return _CACHE[key]


def kernel(embedding, mask, Wq, bq, Wk, bk, Wv, bv):
    embedding = np.asarray(embedding, dtype=np.float32)
    mask = np.asarray(mask, dtype=np.float32)
    Wq = np.asarray(Wq, dtype=np.float32)
    Wk = np.asarray(Wk, dtype=np.float32)
    Wv = np.asarray(Wv, dtype=np.float32)
    bq = np.asarray(bq, dtype=np.float32)
    bk = np.asarray(bk, dtype=np.float32)
    bv = np.asarray(bv, dtype=np.float32)

    _, mask_dt = _mask_cfg()
    MNP = F8NP if mask_dt == "f8" else ml_dtypes.bfloat16
    mscale = MASK_VAL if mask_dt == "f8" else np.float32(-100000.0)

    def pad_w(w, b, extra_one=False):
        wp = np.zeros((EPAD, HV if extra_one else HID), dtype=np.float32)
        wp[:EMB, :HID] = w
        wp[EMB, :HID] = b
        if extra_one:
            wp[EMB, HID] = 1.0
        return wp.astype(F16NP)

    wq_a = pad_w(Wq, bq)
    wk_a = pad_w(Wk, bk)
    wv_a = pad_w(Wv, bv, extra_one=True)

    # Each core's query columns are rotated so its key-half occupies
    # columns 0..KL-1: the device then projects K/V from a prefix of the
    # same embT tile (no separate embTk load) and the host un-rotates the
    # output rows after the gather. half=0 is the identity; half=1 swaps
    # the two halves (an involution).
    in_maps = []
    for c in range(NCORES):
        b, half = divmod(c, 2)
        embT = np.zeros((EPAD, L), dtype=np.float32)
        embT[:EMB] = embedding[b].T
        embT[EMB] = 1.0
        embT16 = embT.astype(F16NP)
        ksl = slice(half * KL, (half + 1) * KL)
        mT = (mask[b].T[ksl, :] * mscale).astype(MNP)
        if half == 1:
            embT16 = np.ascontiguousarray(
                np.concatenate([embT16[:, KL:], embT16[:, :KL]], axis=1))
            mT = np.ascontiguousarray(
                np.concatenate([mT[:, KL:], mT[:, :KL]], axis=1))
        in_maps.append({
            "embT": embT16,
            "wq": wq_a, "wk": wk_a, "wv": wv_a,
            "maskT": mT,
        })

    nc = _get_nc()
    trace = bool(int(os.environ.get("BASS_KERNEL_TRACE", "0")))
    res = run_bass_kernel_spmd(nc, in_maps, core_ids=list(range(NCORES)), trace=trace)
    _CACHE["last_results"] = res

    full = np.empty((B, L, HID), dtype=np.float32)
    for b in range(B):
        r0 = res.results[2 * b]["out"].astype(np.float64)
        r1 = res.results[2 * b + 1]["out"].astype(np.float64)
        r1 = np.concatenate([r1[KL:], r1[:KL]], axis=0)  # un-rotate half=1
        num = r0[:, :HID] + r1[:, :HID]
        den = r0[:, HID:] + r1[:, HID:]
        full[b] = (num / den).astype(np.float32)
    return full


# revision 18
# speedup vs baseline: 1.4032x; 1.0066x over previous
"""Single-head attention (B=4, L=4096, EMB=312, HID=256) on 8 NeuronCores.

Sharding: data-parallel over batch (4) x key-parallel (2) = 8 cores. Each
core handles ALL 4096 queries against its half of the keys and returns the
UNNORMALIZED partial [sum_k p*v | sum_k p] rows; the host combines the two
halves as (o1+o2)/(s1+s2). Key-sharding (vs query-sharding) halves the
duplicated K/V projection work; only the Q projection is duplicated.

Device algorithm (per core):
  - Inputs arrive pre-transposed/padded from the host. emb and W* are fp16
    (values are small-range, so fp16's 11-bit mantissa beats bf16 and loads
    half the bytes of fp32); projections are single-pass fp16 matmuls
    accumulated in fp32 PSUM (1 cycle/row on the PE, same rate as bf16).
  - embT carries a ones-row at index EMB and W* carry the bias in that row,
    so projections fold the bias in. Wv has 2 extra columns: ones (gives the
    softmax row-sum through the P@V matmul) and zero padding (even N).
  - q/k/v are stored as fp32r; QK and PV run single-pass fp32r matmuls
    (1 cycle/row at these tile widths, ~tf32 operand precision).
  - Scores are computed transposed: sT[kl, ql] = kT-chunk^T @ qT, so the
    exp() output is directly the stationary operand for the P@V matmul —
    no on-device transposes anywhere.
  - Mask is host-side transposed and encoded as fp8e4m3 {0, -240}: adding
    -240 to a score makes exp() underflow to exactly 0.0 in fp32, which is
    indistinguishable from the reference's -1e5 (no row is fully masked).
    The DVE applies it additively to the score PSUM; exp() on ACT.
  - DMA discipline: the TimelineSim charges ~565-667ns of sequencer time
    per dma_start on the SP/ACT/DVE rings plus a shared-HWDGE hold, so
    transfers are batched: 12 emb block loads + 3 weight loads + 8 mask
    loads (one per query tile, on the gpsimd/SWDGE ring which bypasses
    HWDGE) + 8 output stores. ~31 DMAs total vs 232 in the bf16x2 version.
  - The raw partials (P@V columns + row-sum column) go back to the host,
    which normalizes after combining the key-halves.

Env overrides (debug): BASS_KERNEL_MASK_RING=gpsimd|scalar,
BASS_KERNEL_MASK_DT=f8|bf16.
"""
import os

import numpy as np
import ml_dtypes

import concourse.bacc as bacc
import concourse.tile as tile
from concourse import mybir, bass2jax
from concourse.bass_utils import run_bass_kernel_spmd

# Debug aid (opt-in): surface real compile errors from the PJRT compile
# hook, which the C++ bridge otherwise swallows.
if os.environ.get("BASS_KERNEL_DEBUG"):
    import functools as _ft
    import traceback as _tb
    _orig_hook = bass2jax.neuronx_cc_hook
    @_ft.wraps(_orig_hook)
    def _dbg_hook(*args, **kwargs):
        try:
            return _orig_hook(*args, **kwargs)
        except BaseException:
            _tb.print_exc()
            raise
    bass2jax.neuronx_cc_hook = _dbg_hook

EMB, HID, B, L = 312, 256, 4, 4096
NCORES = 8
P = 128
KL = L // 2            # key rows per core (key-parallel halves)
EPAD = 384             # emb dim padded to 3 partition chunks; row EMB is the ones-row
HV = HID + 2           # v columns: HID values | ones | zero pad (even N)
QT = 512               # ql tile width (PSUM bank = 512 fp32)
NKC = KL // P          # 16 kl chunks per core
NQT = L // QT          # 8 ql tiles per core (all queries)
NKT = KL // QT         # 4 l tiles for the k projection
MASK_VAL = np.float32(-240.0)   # exactly representable in fp8e4m3

F32 = mybir.dt.float32
F16 = mybir.dt.float16
F32R = mybir.dt.float32r
F8 = mybir.dt.float8e4
BF16 = mybir.dt.bfloat16
F16NP = np.float16
F8NP = ml_dtypes.float8_e4m3

_CACHE = {}


def _mask_cfg():
    ring = os.environ.get("BASS_KERNEL_MASK_RING", "gpsimd")
    dt = os.environ.get("BASS_KERNEL_MASK_DT", "f8")
    return ring, dt


def _build():
    mask_ring, mask_dt = _mask_cfg()
    MDT = F8 if mask_dt == "f8" else BF16

    nc = bacc.Bacc(None)

    embT = nc.dram_tensor("embT", [EPAD, L], F16, kind="ExternalInput")
    wq = nc.dram_tensor("wq", [EPAD, HID], F16, kind="ExternalInput")
    wk = nc.dram_tensor("wk", [EPAD, HID], F16, kind="ExternalInput")
    wv = nc.dram_tensor("wv", [EPAD, HV], F16, kind="ExternalInput")
    maskT = nc.dram_tensor("maskT", [KL, L], MDT, kind="ExternalInput")
    out = nc.dram_tensor("out", [L, HID + 1], F32, kind="ExternalOutput")

    with tile.TileContext(nc) as tc:
        with (
            tc.tile_pool(name="big", bufs=1) as big,
            tc.tile_pool(name="wp", bufs=1) as wp,
            tc.tile_pool(name="mt", bufs=2) as mtp,
            tc.tile_pool(name="pt", bufs=6) as ptp,
            tc.tile_pool(name="fin", bufs=2) as fin,
            tc.tile_pool(name="ps_st", bufs=4, space="PSUM") as ps_st,
            tc.tile_pool(name="ps_pv", bufs=1, space="PSUM") as ps_pv,
        ):
            # ---- input loads. Weight tensors ride the ACT ring; emb blocks
            # ride the SP ring, lowest columns first so the first projection
            # matmuls start a couple of us in. Each DMA covers all 3
            # emb-chunks of its column block (partition p reads rows
            # {p, 128+p, 256+p}).
            wk_t = wp.tile([P, 3, HID], F16, name="wk_t")
            wv_t = wp.tile([P, 3, HV], F16, name="wv_t")
            wq_t = wp.tile([P, 3, HID], F16, name="wq_t")
            # wk leads on the SP ring (lowest fixed issue cost) since the
            # very first projection matmul needs it; wv/wq ride the ACT ring
            # in parallel.
            nc.sync.dma_start(
                out=wk_t, in_=wk[:, :].rearrange("(c p) n -> p c n", p=P))
            for t, d in ((wv_t, wv), (wq_t, wq)):
                nc.scalar.dma_start(
                    out=t, in_=d[:, :].rearrange("(c p) n -> p c n", p=P))

            # The host rotates each core's query columns so its key-half
            # occupies columns 0..KL-1 (undone host-side on the output), so
            # the K/V projections read a PREFIX of embT and no separate
            # embTk load is needed.
            embT_t = big.tile([P, 3, L], F16, name="embT_t")
            for b0 in range(0, L, QT):
                if b0 == 0:
                    # Split the first block per emb-chunk so the very first
                    # projection matmul (which only needs chunk 0) starts
                    # ~1us earlier.
                    for cch in range(3):
                        nc.sync.dma_start(
                            out=embT_t[:, cch, 0:QT],
                            in_=embT[cch * P:(cch + 1) * P, 0:QT],
                        )
                else:
                    nc.sync.dma_start(
                        out=embT_t[:, :, b0:b0 + QT],
                        in_=embT[:, b0:b0 + QT].rearrange("(c p) n -> p c n", p=P),
                    )

            kT_r = big.tile([P, 2, KL], F32R, name="kT_r")
            qT_r = big.tile([P, 2, L], F32R, name="qT_r")
            v_r = big.tile([P, NKC, HV], F32R, name="v_r")

            # ---- projections (single-pass fp16, fp32 PSUM accumulate).
            # q/k in [h(part), hc, l(free)] layout; v in [kl(part), klc, h].
            # k/q PSUM->SBUF copies go to the DVE and v copies to ACT so the
            # copy work never gates the PE during the projection phase.
            def emit_kq(hc, lt, which):
                ps = ps_st.tile([P, QT], F32, name="st", tag="st")
                w, dst = (wk_t, kT_r) if which == "k" else (wq_t, qT_r)
                for ei in range(3):
                    nc.tensor.matmul(
                        ps,
                        lhsT=w[:, ei, hc * P:(hc + 1) * P],
                        rhs=embT_t[:, ei, lt * QT:(lt + 1) * QT],
                        start=(ei == 0), stop=(ei == 2),
                    )
                nc.vector.tensor_copy(dst[:, hc, lt * QT:(lt + 1) * QT], ps)

            def emit_v(kc):
                ps = ps_st.tile([P, QT], F32, name="st", tag="st")
                for ei in range(3):
                    nc.tensor.matmul(
                        ps[:, :HV],
                        lhsT=embT_t[:, ei, kc * P:(kc + 1) * P],
                        rhs=wv_t[:, ei, :],
                        start=(ei == 0), stop=(ei == 2),
                    )
                nc.scalar.copy(out=v_r[:, kc, :], in_=ps[:, :HV])

            kq_tiles = [("k", hc, lt) for lt in range(NKT) for hc in range(2)]
            kq_tiles += [("q", hc, lt) for lt in range(NQT) for hc in range(2)]
            vi = 0
            for i, (which, hc, lt) in enumerate(kq_tiles):
                emit_kq(hc, lt, which)
                want_v = ((i + 1) * NKC) // len(kq_tiles)
                while vi < want_v:
                    emit_v(vi)
                    vi += 1
            while vi < NKC:
                emit_v(vi)
                vi += 1

            # ---- attention
            # Uniform lag-2 software pipeline carried ACROSS ql-tile
            # boundaries: chunk kc's P@V matmuls are emitted after chunk
            # kc+2's QK matmuls (even across ql tiles), so the PE always has
            # ~2 tiles of independent work in program order while the DVE
            # mask-add + ACT exp + pv-bank WAR release of the current chunk
            # are still in flight. One mask DMA per ql tile ([2048, 512]
            # block, gpsimd/SWDGE ring) with 3 buffers -> 2-deep prefetch.
            # pv PSUM banks are reused every tile; the staging copies (DVE)
            # are emitted at the kc==15 flush, which under lag-2 lands
            # between the next tile's mask-adds early enough that the new
            # accumulation's per-bank WAR is satisfied before the PE gets
            # there. pvs allocation for a tile happens at its kc==0 flush,
            # after those copies.
            mask_dma = nc.gpsimd if mask_ring == "gpsimd" else nc.scalar
            from collections import deque

            # pv accumulators: ONE PSUM tile [P, 4, 512] so each j block
            # owns exactly one 2KB bank (matmul outputs stay bank-local) and
            # the output staging is a single DVE copy instead of four.
            pvs_box = [None]
            LAG = 3

            def emit_pv(oqt, kc, ptile):
                if kc == 0:
                    pvs_box[0] = ps_pv.tile([P, 4, QT], F32, name="pv", tag="pv")
                pv = pvs_box[0]
                for j in range(4):
                    nc.tensor.matmul(
                        pv[:, j, :HV],
                        lhsT=ptile[:, j * P:(j + 1) * P],
                        rhs=v_r[:, kc, :],
                        start=(kc == 0), stop=(kc == NKC - 1),
                    )
                if kc == NKC - 1:
                    # Ship the unnormalized partial [sum p*v | sum p]; the
                    # host divides after combining the two key-halves. For
                    # the last ql tile the copies+stores go per-j so the
                    # kernel tail isn't serialized behind one fused copy.
                    if oqt == NQT - 1:
                        # copies split DVE/ACT, stores fan across 4 rings so
                        # nothing serializes on one sequencer at the drain.
                        rings = (nc.sync, nc.scalar, nc.gpsimd, nc.sync)
                        for j in range(4):
                            otj = fin.tile([P, HID + 1], F32, name="otj",
                                           tag=f"otj{j}")
                            if j % 2:
                                nc.scalar.copy(out=otj, in_=pv[:, j, :HID + 1])
                            else:
                                nc.vector.tensor_copy(otj, pv[:, j, :HID + 1])
                            r0 = (oqt * 4 + j) * P
                            rings[j].dma_start(out=out[r0:r0 + P, :], in_=otj)
                    else:
                        ot = fin.tile([P, 4, HID + 1], F32, name="ot", tag="ot")
                        nc.vector.tensor_copy(ot, pv[:, :, :HID + 1])
                        nc.sync.dma_start(
                            out=out[oqt * QT:(oqt + 1) * QT, :].rearrange(
                                "(j p) n -> p j n", p=P),
                            in_=ot,
                        )

            pending = deque()  # (qt, kc, p-tile) awaiting PV emission
            for qt in range(NQT):
                qsl = slice(qt * QT, (qt + 1) * QT)
                mk = mtp.tile([P, NKC, QT], MDT, name="mk", tag="mk")
                # The first two mask loads ride the SP ring, whose in-order
                # program puts them AFTER the embT blocks — otherwise the
                # Pool ring issues them at t=0 and their transfers preempt
                # the startup emb loads on the shared DMA engines. Later
                # tiles (gated by the 2-buffer pool anyway) use the Pool
                # ring, keeping the SP ring free for output stores.
                ring = nc.sync if qt < 2 else mask_dma
                ring.dma_start(
                    out=mk, in_=maskT[:, qsl].rearrange("(c p) n -> p c n", p=P))
                for kc in range(NKC):
                    st = ps_st.tile([P, QT], F32, name="st", tag="st")
                    for hc in range(2):
                        nc.tensor.matmul(
                            st,
                            lhsT=kT_r[:, hc, kc * P:(kc + 1) * P],
                            rhs=qT_r[:, hc, qsl],
                            start=(hc == 0), stop=(hc == 1),
                        )
                    if len(pending) == LAG:
                        emit_pv(*pending.popleft())
                    nc.vector.tensor_tensor(
                        out=st, in0=st, in1=mk[:, kc, :], op=mybir.AluOpType.add)
                    pt_ = ptp.tile([P, QT], F32R, name="pt", tag="pt")
                    nc.scalar.activation(
                        out=pt_, in_=st, func=mybir.ActivationFunctionType.Exp)
                    pending.append((qt, kc, pt_))
            while pending:
                emit_pv(*pending.popleft())
    nc.finalize()
    return nc


def _get_nc():
    key = "nc_turbo_" + "_".join(_mask_cfg())
    if key not in _CACHE:
        _CACHE[key] = _build()
    return _CACHE[key]


def kernel(embedding, mask, Wq, bq, Wk, bk, Wv, bv):
    embedding = np.asarray(embedding, dtype=np.float32)
    mask = np.asarray(mask, dtype=np.float32)
    Wq = np.asarray(Wq, dtype=np.float32)
    Wk = np.asarray(Wk, dtype=np.float32)
    Wv = np.asarray(Wv, dtype=np.float32)
    bq = np.asarray(bq, dtype=np.float32)
    bk = np.asarray(bk, dtype=np.float32)
    bv = np.asarray(bv, dtype=np.float32)

    _, mask_dt = _mask_cfg()
    MNP = F8NP if mask_dt == "f8" else ml_dtypes.bfloat16
    mscale = MASK_VAL if mask_dt == "f8" else np.float32(-100000.0)

    def pad_w(w, b, extra_one=False):
        wp = np.zeros((EPAD, HV if extra_one else HID), dtype=np.float32)
        wp[:EMB, :HID] = w
        wp[EMB, :HID] = b
        if extra_one:
            wp[EMB, HID] = 1.0
        return wp.astype(F16NP)

    wq_a = pad_w(Wq, bq)
    wk_a = pad_w(Wk, bk)
    wv_a = pad_w(Wv, bv, extra_one=True)

    # Each core's query columns are rotated so its key-half occupies
    # columns 0..KL-1: the device then projects K/V from a prefix of the
    # same embT tile (no separate embTk load) and the host un-rotates the
    # output rows after the gather. half=0 is the identity; half=1 swaps
    # the two halves (an involution).
    in_maps = []
    for c in range(NCORES):
        b, half = divmod(c, 2)
        embT = np.zeros((EPAD, L), dtype=np.float32)
        embT[:EMB] = embedding[b].T
        embT[EMB] = 1.0
        embT16 = embT.astype(F16NP)
        ksl = slice(half * KL, (half + 1) * KL)
        mT = (mask[b].T[ksl, :] * mscale).astype(MNP)
        if half == 1:
            embT16 = np.ascontiguousarray(
                np.concatenate([embT16[:, KL:], embT16[:, :KL]], axis=1))
            mT = np.ascontiguousarray(
                np.concatenate([mT[:, KL:], mT[:, :KL]], axis=1))
        in_maps.append({
            "embT": embT16,
            "wq": wq_a, "wk": wk_a, "wv": wv_a,
            "maskT": mT,
        })

    nc = _get_nc()
    trace = bool(int(os.environ.get("BASS_KERNEL_TRACE", "0")))
    res = run_bass_kernel_spmd(nc, in_maps, core_ids=list(range(NCORES)), trace=trace)
    _CACHE["last_results"] = res

    full = np.empty((B, L, HID), dtype=np.float32)
    for b in range(B):
        r0 = res.results[2 * b]["out"].astype(np.float64)
        r1 = res.results[2 * b + 1]["out"].astype(np.float64)
        r1 = np.concatenate([r1[KL:], r1[:KL]], axis=0)  # un-rotate half=1
        num = r0[:, :HID] + r1[:, :HID]
        den = r0[:, HID:] + r1[:, HID:]
        full[b] = (num / den).astype(np.float32)
    return full
